# revision 1
# baseline (speedup 1.0000x reference)
"""Trainium2 Bass kernel for nn_MultiHeadAttention_412316861010.

Sharding: batch x head-group over 8 cores (core c -> batch c//4, heads
(c%4)*4 .. +4). All activations/weights stream in bf16 (halves HBM
traffic vs fp32). Per core:

  Phase A: Q/K/V projections as bf16 matmuls accumulating in PSUM.
    qhT/khT stored [128, 2*S] bf16 (column block fc holds heads 2fc at
    partitions 0:64 and 2fc+1 at 64:128). V stored per key-chunk
    [128 keys, 4 heads x 65] with a ones column per head (PV then
    yields the softmax denominator for free in ctx row 64).
  Phase B: attention, query-block (512) outer, head-pair inner. Per
    (sq, fc, sk): the two heads' QK^T matmuls run concurrently in the
    two 64-row PE tile groups into one [128,1024] PSUM tile; one wide
    exp on ACT; two mask multiplies on DVE; two PV matmuls (M=65)
    accumulate ctx. Denominator reciprocal via the fast custom-DVE
    approx (the exact DVE reciprocal costs ~6.5us per [1,512] row),
    gpsimd partition-broadcast, DVE normalize into bf16 ctx_sb.
  Phase C: per query block (overlapped with the next block's
    attention): output projection and bf16 partial-out DMA.

The host sums the 4 partial projections per batch and adds the output
bias. Self-contained: hardcodes all shapes from the problem spec.
"""
import numpy as np
import ml_dtypes

import concourse.bass as bass
import concourse.mybir as mybir
import concourse.tile as tile
from concourse import bacc
from concourse.bass_utils import run_bass_kernel_spmd

B, S, E, H = 2, 2048, 1024, 16
D = E // H            # 64 head dim
NCORES = 8
HPC = 4               # heads per core
FW = HPC * D          # 256 features per core
F32 = mybir.dt.float32
BF16 = mybir.dt.bfloat16

Exp = mybir.ActivationFunctionType.Exp

NSQ = 4               # query blocks
SQW = S // NSQ        # 512 queries per block


def build_nc():
    nc = bacc.Bacc("TRN2", target_bir_lowering=False, debug=False, num_devices=NCORES)

    xqt = nc.dram_tensor("xqt", [E, S], BF16, kind="ExternalInput")
    xkt = nc.dram_tensor("xkt", [E, S], BF16, kind="ExternalInput")
    xvt = nc.dram_tensor("xvt", [E, S], BF16, kind="ExternalInput")
    wq = nc.dram_tensor("wq", [E, FW], BF16, kind="ExternalInput")
    wk = nc.dram_tensor("wk", [E, FW], BF16, kind="ExternalInput")
    wv = nc.dram_tensor("wv", [E, FW], BF16, kind="ExternalInput")
    bq = nc.dram_tensor("bq", [1, FW], BF16, kind="ExternalInput")
    bk = nc.dram_tensor("bk", [1, FW], BF16, kind="ExternalInput")
    bv = nc.dram_tensor("bv", [1, FW], BF16, kind="ExternalInput")
    wo = nc.dram_tensor("wo", [FW, E], BF16, kind="ExternalInput")
    maskt = nc.dram_tensor("maskt", [S, S], BF16, kind="ExternalInput")
    out = nc.dram_tensor("out", [S, E], BF16, kind="ExternalOutput")

    with tile.TileContext(nc) as tc:
        with tc.tile_pool(name="per", bufs=1) as per, \
             tc.tile_pool(name="xp", bufs=3) as xp, \
             tc.tile_pool(name="ep", bufs=3) as ep, \
             tc.tile_pool(name="atp", bufs=64) as atp, \
             tc.tile_pool(name="mp", bufs=3) as mp, \
             tc.tile_pool(name="bcp", bufs=2) as bcp, \
             tc.tile_pool(name="outp", bufs=2) as outp:

            # ---- persistent SBUF ----
            wq_sb = per.tile([128, 8 * FW], BF16, name="wq_sb")
            wk_sb = per.tile([128, 8 * FW], BF16, name="wk_sb")
            wv_sb = per.tile([128, 8 * FW], BF16, name="wv_sb")
            wo_sb = per.tile([128, 2 * E], BF16, name="wo_sb")
            bq_sb = per.tile([1, FW], BF16, name="bq_sb")
            bk_sb = per.tile([1, FW], BF16, name="bk_sb")
            bv_sb = per.tile([1, FW], BF16, name="bv_sb")
            qht_sb = per.tile([128, 2 * S], BF16, name="qht_sb")
            kht_sb = per.tile([128, 2 * S], BF16, name="kht_sb")
            vh_sb = per.tile([128, 16 * 260], BF16, name="vh_sb")
            ctx_sb = per.tile([128, 2 * S], BF16, name="ctx_sb")
            ones_f = per.tile([1, 512], F32, name="ones_f")
            ones_b = per.tile([1, 512], BF16, name="ones_b")
            bv2_sb = per.tile([1, 512], BF16, name="bv2_sb")
            bv3_sb = per.tile([1, 512], BF16, name="bv3_sb")
            # rotating mask tiles: one [128, 16*SQW] slice per query block
            # (3 slots; block 3's tile reuses block 0's slot)
            mtiles = {}

            nc.vector.memset(ones_f[:], 1.0)
            nc.vector.tensor_copy(ones_b[:], ones_f[:])
            nc.vector.memset(vh_sb[:], 1.0)

            wdma = {"q": (wq_sb, wq, bq_sb, bq), "k": (wk_sb, wk, bk_sb, bk),
                    "v": (wv_sb, wv, bv_sb, bv)}

            def load_w(nm):
                w_sb_, w_, b_sb_, b_ = wdma[nm]
                nc.sync.dma_start(w_sb_[:].rearrange("p (c n) -> p c n", c=8),
                                  w_.ap().rearrange("(c p) n -> p c n", p=128))
                nc.sync.dma_start(b_sb_[:], b_.ap())

            def load_mask_qb(qb):
                mtiles[qb] = mp.tile([128, 16 * SQW], BF16, tag="mask",
                                     name=f"mask{qb}")
                for c in range(16):
                    nc.sync.dma_start(
                        mtiles[qb][:, c * SQW:(c + 1) * SQW],
                        maskt.ap()[c * 128:(c + 1) * 128,
                                   qb * SQW:(qb + 1) * SQW])

            # ================= phase A: projections =================
            # DMA order: q/k weights + x first (QK attention can start right
            # after the k projection), masks + v after, wo last.
            load_w("q")
            load_w("k")

            with tc.tile_pool(name="pp", bufs=1, space="PSUM") as pp:
                # q and k projections: qhT/khT [256, S] bf16 as
                # [128, fc * S] (fc = feature chunk of 128 = 2 heads)
                for nm, xdram, w_sb, b_sb, dst in (
                        ("q", xqt, wq_sb, bq_sb, qht_sb),
                        ("k", xkt, wk_sb, bk_sb, kht_sb)):
                    accs = [pp.tile([128, 512], F32, tag=f"acc{i}", name=f"acc_{nm}{i}")
                            for i in range(8)]
                    for e in range(8):
                        x_t = xp.tile([128, S], BF16, tag="x", name=f"x_{nm}{e}")
                        nc.sync.dma_start(x_t[:], xdram.ap()[e * 128:(e + 1) * 128, :])
                        for fc in range(2):
                            for sq in range(4):
                                nc.tensor.matmul(
                                    accs[fc * 4 + sq][:],
                                    w_sb[:, e * FW + fc * 128: e * FW + fc * 128 + 128],
                                    x_t[:, sq * 512:(sq + 1) * 512],
                                    start=(e == 0), stop=False)
                    for fc in range(2):
                        for sq in range(4):
                            a = accs[fc * 4 + sq]
                            nc.tensor.matmul(a[:], b_sb[0:1, fc * 128:fc * 128 + 128],
                                             ones_b[0:1, :], start=False, stop=True)
                            dstap = dst[:, fc * S + sq * 512: fc * S + sq * 512 + 512]
                            if sq % 2 == 0:
                                nc.scalar.copy(dstap, a[:])
                            else:
                                nc.vector.tensor_copy(dstap, a[:])
                # mask tiles for query block 0 only (2 MB) — attention on
                # block 0 can start as soon as these + q/k are in
                load_mask_qb(0)
                load_w("v")
                nc.vector.tensor_copy(bv2_sb[0:1, 0:128], bv_sb[0:1, 0:128])
                nc.vector.tensor_copy(bv2_sb[0:1, 128:256], bv_sb[0:1, 0:128])
                nc.vector.tensor_copy(bv2_sb[0:1, 256:384], bv_sb[0:1, 0:128])
                nc.vector.tensor_copy(bv2_sb[0:1, 384:512], bv_sb[0:1, 0:128])
                nc.vector.tensor_copy(bv3_sb[0:1, 0:128], bv_sb[0:1, 128:256])
                nc.vector.tensor_copy(bv3_sb[0:1, 128:256], bv_sb[0:1, 128:256])
                nc.vector.tensor_copy(bv3_sb[0:1, 256:384], bv_sb[0:1, 128:256])
                nc.vector.tensor_copy(bv3_sb[0:1, 384:512], bv_sb[0:1, 128:256])

            # ============ phase B + C: attention, query-block outer ============
            # Software-pipelined head of the schedule: both head-pairs' QK,
            # exp and mask-mul for query block 0 are emitted BEFORE the V
            # projection (their PV matmuls are deferred via a 64-slot at
            # pool), so the scalar engine starts the exp stream as soon as
            # the k projection lands instead of waiting for xv. The V
            # projection itself is split into two 4-bank passes (one per
            # head-pair) that live in the shared attention PSUM pool.
            # Deferred emission elsewhere: the normalize tensor-muls of
            # block i are emitted a few iterations into block i+1 (DVE never
            # stalls on the gpsimd broadcast), and phase C of query block sq
            # is emitted mid-attention of block sq+1 (its PSUM evacuations
            # never block the strict-FIFO ACT/DVE queues, and the extra
            # matmuls keep the PE warm).
            with tc.tile_pool(name="sp", bufs=2, space="PSUM") as sp, \
                 tc.tile_pool(name="cp", bufs=4, space="PSUM") as cp:
                pending_tt = []    # closures: normalize TT muls of prev block
                pending_c = []     # closures: phase C of prev query block

                def v_pass(fcx):
                    # V projection for heads 2*fcx, 2*fcx+1: 16 key chunks x
                    # 128 features in 4 one-bank accumulators.
                    accs = [cp.tile([128, 512], F32, tag="ps1", name=f"vacc{fcx}_{j}")
                            for j in range(4)]
                    bvt = bv2_sb if fcx == 0 else bv3_sb
                    for j in range(4):
                        nc.tensor.matmul(accs[j][:], ones_b[0:1, 0:128], bvt[0:1, :],
                                         start=True, stop=False, skip_group_check=True)
                    for e in range(8):
                        x_t = xp.tile([128, S], BF16, tag="x", name=f"x_v{fcx}_{e}")
                        nc.sync.dma_start(x_t[:], xvt.ap()[e * 128:(e + 1) * 128, :])
                        for j in range(4):
                            for m in range(4):
                                sk = j * 4 + m
                                nc.tensor.matmul(
                                    accs[j][:, m * 128:(m + 1) * 128],
                                    x_t[:, sk * 128:(sk + 1) * 128],
                                    wv_sb[:, e * FW + fcx * 128: e * FW + fcx * 128 + 128],
                                    start=False, stop=(e == 7), skip_group_check=True)
                    for j in range(4):
                        for m in range(4):
                            sk = j * 4 + m
                            dstap = (vh_sb[:, sk * 260 + 2 * fcx * 65:
                                           sk * 260 + 2 * fcx * 65 + 130]
                                     .rearrange("p (h z) -> p h z", h=2)[:, :, 0:D])
                            srcap = (accs[j][:, m * 128:(m + 1) * 128]
                                     .rearrange("p (h z) -> p h z", h=2))
                            if m % 2 == 0:
                                nc.scalar.copy(dstap, srcap)
                            else:
                                nc.vector.tensor_copy(dstap, srcap)

                def phase_c(sq):
                    q0 = sq * SQW
                    for qc in range(4):
                        r0 = q0 + qc * 128
                        o_t = outp.tile([128, E], BF16, tag="o", name=f"o{sq}_{qc}")
                        for i in range(2):
                            op_ps = cp.tile([128, 512], F32, tag="ps1",
                                            name=f"op{sq}_{qc}_{i}")
                            for fcc in range(2):
                                nc.tensor.matmul(
                                    op_ps[:],
                                    ctx_sb[:, fcc * S + r0: fcc * S + r0 + 128],
                                    wo_sb[:, fcc * E + i * 512: fcc * E + i * 512 + 512],
                                    start=(fcc == 0), stop=(fcc == 1))
                            dstap = o_t[:, i * 512:(i + 1) * 512]
                            if (qc + i) % 2 == 0:
                                nc.scalar.copy(dstap, op_ps[:])
                            else:
                                nc.vector.tensor_copy(dstap, op_ps[:])
                        nc.sync.dma_start(out.ap()[r0:r0 + 128, :], o_t[:])

                def qk_exp_mul(sq, fc, sk):
                    """Emit the QK pair, exp, and the two mask-muls; return
                    the two at tiles (PV may be emitted later)."""
                    q0 = sq * SQW
                    sc = sp.tile([128, 1024], F32, tag="sc", name=f"sc{sq}_{fc}_{sk}")
                    for po, half in ((0, 0), (64, 1)):
                        nc.tensor.matmul(
                            sc[:, half * 512: half * 512 + 512],
                            kht_sb[po:po + 64,
                                   fc * S + sk * 128: fc * S + sk * 128 + 128],
                            qht_sb[po:po + 64, fc * S + q0: fc * S + q0 + SQW],
                            start=True, stop=True)
                    ex = ep.tile([128, 1024], BF16, tag="ex", name=f"ex{sq}_{fc}_{sk}")
                    nc.scalar.activation(ex[:], sc[:], Exp, scale=0.125)
                    mchunk = mtiles[sq][:, sk * SQW:(sk + 1) * SQW]
                    ats = []
                    for half in (0, 1):
                        at = atp.tile([128, SQW], BF16, tag="at",
                                      name=f"at{sq}_{fc}_{sk}_{half}")
                        nc.vector.tensor_mul(
                            at[:], ex[:, half * 512: half * 512 + 512], mchunk)
                        ats.append(at)
                    return ats

                def pv(fc, sk, half, at, ctx):
                    h = fc * 2 + half
                    nc.tensor.matmul(
                        ctx[:],
                        vh_sb[:, sk * 260 + h * 65: sk * 260 + h * 65 + 65],
                        at[:],
                        start=(sk == 0), stop=(sk == 15), skip_group_check=True)

                def normalize(sq, fc, po, ctx, defer):
                    q0 = sq * SQW
                    r_f = bcp.tile([1, SQW], F32, tag="r_f", name=f"rf{sq}_{fc}_{po}")
                    nc.vector.tensor_copy(r_f[:], ctx[64:65, :])
                    r_rec = bcp.tile([1, SQW], F32, tag="r_rec",
                                     name=f"rr{sq}_{fc}_{po}")
                    nc.vector.reciprocal_approx_fast(r_rec[:], r_f[:])
                    bc_t = bcp.tile([64, SQW], F32, tag="bc", name=f"bc{sq}_{fc}_{po}")
                    nc.gpsimd.partition_broadcast(bc_t[:], r_rec[:])

                    def tt():
                        nc.vector.tensor_mul(
                            ctx_sb[po:po + 64, fc * S + q0: fc * S + q0 + SQW],
                            ctx[0:64, :], bc_t[:])
                    if defer:
                        pending_tt.append(tt)
                    else:
                        tt()

                # ---- pipelined first query block ----
                ats0 = {}
                for fc in range(2):
                    for sk in range(16):
                        ats0[(fc, sk)] = qk_exp_mul(0, fc, sk)
                v_pass(0)
                load_mask_qb(1)
                ctxA = cp.tile([65, SQW], F32, tag="ps1", name="ctxA0_0")
                ctxB = cp.tile([65, SQW], F32, tag="ps1", name="ctxB0_0")
                for sk in range(16):
                    for half, ctx in ((0, ctxA), (1, ctxB)):
                        pv(0, sk, half, ats0[(0, sk)][half], ctx)
                normalize(0, 0, 0, ctxA, defer=False)
                normalize(0, 0, 64, ctxB, defer=False)
                v_pass(1)
                nc.sync.dma_start(wo_sb[:].rearrange("p (c n) -> p c n", c=2),
                                  wo.ap().rearrange("(c p) n -> p c n", p=128))
                load_mask_qb(2)
                load_mask_qb(3)
                ctxA = cp.tile([65, SQW], F32, tag="ps1", name="ctxA0_1")
                ctxB = cp.tile([65, SQW], F32, tag="ps1", name="ctxB0_1")
                for sk in range(16):
                    for half, ctx in ((0, ctxA), (1, ctxB)):
                        pv(1, sk, half, ats0[(1, sk)][half], ctx)
                normalize(0, 1, 0, ctxA, defer=True)
                normalize(0, 1, 64, ctxB, defer=True)
                pending_c.append(lambda: phase_c(0))
                del ats0

                # ---- steady-state blocks ----
                for sq in range(1, NSQ):
                    for fc in range(2):
                        ctxA = cp.tile([65, SQW], F32, tag="ps1", name=f"ctxA{sq}_{fc}")
                        ctxB = cp.tile([65, SQW], F32, tag="ps1", name=f"ctxB{sq}_{fc}")
                        for sk in range(16):
                            atA, atB = qk_exp_mul(sq, fc, sk)
                            pv(fc, sk, 0, atA, ctxA)
                            pv(fc, sk, 1, atB, ctxB)
                            if sk == 2:
                                for f in pending_tt:
                                    f()
                                pending_tt.clear()
                            if sk == 6:
                                for f in pending_c:
                                    f()
                                pending_c.clear()
                        normalize(sq, fc, 0, ctxA, defer=True)
                        normalize(sq, fc, 64, ctxB, defer=True)
                    pending_c.append(lambda sq=sq: phase_c(sq))

                for f in pending_tt:
                    f()
                for f in pending_c:
                    f()

    nc.compile()
    return nc


_CACHE = {}


def _get_nc():
    if "nc" not in _CACHE:
        _CACHE["nc"] = build_nc()
    return _CACHE["nc"]


def make_in_maps(q, k, v, mask, Wqkv, bqkv, Wout):
    bf = ml_dtypes.bfloat16
    maskt = np.ascontiguousarray(mask[0, 0].T).astype(bf)
    qT = [np.ascontiguousarray(np.asarray(q[b]).T).astype(bf) for b in range(B)]
    kT = [np.ascontiguousarray(np.asarray(k[b]).T).astype(bf) for b in range(B)]
    vT = [np.ascontiguousarray(np.asarray(v[b]).T).astype(bf) for b in range(B)]
    Wqkv = np.asarray(Wqkv)
    bqkv = np.asarray(bqkv)
    Wout = np.asarray(Wout)
    in_maps = []
    for c in range(NCORES):
        b = c // 4
        h0 = (c % 4) * HPC
        fsl = slice(h0 * D, (h0 + HPC) * D)
        in_maps.append({
            "xqt": qT[b],
            "xkt": kT[b],
            "xvt": vT[b],
            "wq": np.ascontiguousarray(Wqkv[:, 0:E][:, fsl]).astype(bf),
            "wk": np.ascontiguousarray(Wqkv[:, E:2 * E][:, fsl]).astype(bf),
            "wv": np.ascontiguousarray(Wqkv[:, 2 * E:3 * E][:, fsl]).astype(bf),
            "bq": np.ascontiguousarray(bqkv[0:E][fsl]).reshape(1, FW).astype(bf),
            "bk": np.ascontiguousarray(bqkv[E:2 * E][fsl]).reshape(1, FW).astype(bf),
            "bv": np.ascontiguousarray(bqkv[2 * E:3 * E][fsl]).reshape(1, FW).astype(bf),
            "wo": np.ascontiguousarray(Wout[fsl, :]).astype(bf),
            "maskt": maskt,
        })
    return in_maps


def gather(results, bout):
    out = np.empty((B, S, E), np.float32)
    for b in range(B):
        acc = results[4 * b]["out"].astype(np.float32)
        for c in range(4 * b + 1, 4 * b + 4):
            acc += results[c]["out"].astype(np.float32)
        out[b] = acc + np.asarray(bout)[None, :]
    return out


def kernel(q, k, v, mask, Wqkv, bqkv, Wout, bout):
    nc = _get_nc()
    in_maps = make_in_maps(q, k, v, mask, Wqkv, bqkv, Wout)
    res = run_bass_kernel_spmd(nc, in_maps, core_ids=list(range(NCORES)))
    return gather(res.results, np.asarray(bout))



# revision 5
# speedup vs baseline: 1.9787x; 1.9787x over previous
"""Trainium2 Bass kernel for nn_MultiHeadAttention_412316861010.

Sharding: batch x head-group over 8 cores (core c -> batch c//4, heads
(c%4)*4 .. +4). All activations/weights stream in bf16 (halves HBM
traffic vs fp32). Per core:

  Phase A: Q/K/V projections as bf16 matmuls accumulating in PSUM.
    qhT/khT stored [128, 2*S] bf16 (column block fc holds heads 2fc at
    partitions 0:64 and 2fc+1 at 64:128). V stored per key-chunk
    [128 keys, 4 heads x 65] with a ones column per head (PV then
    yields the softmax denominator for free in ctx row 64).
  Phase B: attention, query-block (512) outer, head-pair inner. Per
    (sq, fc, sk): the two heads' QK^T matmuls run concurrently in the
    two 64-row PE tile groups into one [128,1024] PSUM tile; one wide
    exp on ACT; two mask multiplies on DVE; two PV matmuls (M=65)
    accumulate ctx. Denominator reciprocal via the fast custom-DVE
    approx (the exact DVE reciprocal costs ~6.5us per [1,512] row),
    gpsimd partition-broadcast, DVE normalize into bf16 ctx_sb.
  Phase C: per query block (overlapped with the next block's
    attention): output projection and bf16 partial-out DMA.

The host sums the 4 partial projections per batch and adds the output
bias. Self-contained: hardcodes all shapes from the problem spec.
"""
import numpy as np
import ml_dtypes

import concourse.bass as bass
import concourse.mybir as mybir
import concourse.tile as tile
from concourse import bacc

B, S, E, H = 2, 2048, 1024, 16
D = E // H            # 64 head dim
NCORES = 8
HPC = 4               # heads per core
FW = HPC * D          # 256 features per core
F32 = mybir.dt.float32
BF16 = mybir.dt.bfloat16

Exp = mybir.ActivationFunctionType.Exp

NSQ = 4               # query blocks
SQW = S // NSQ        # 512 queries per block


def build_nc():
    nc = bacc.Bacc("TRN2", target_bir_lowering=False, debug=False, num_devices=NCORES)

    xqt = nc.dram_tensor("xqt", [E, S], BF16, kind="ExternalInput")
    xkt = nc.dram_tensor("xkt", [E, S], BF16, kind="ExternalInput")
    xvt = nc.dram_tensor("xvt", [E, S], BF16, kind="ExternalInput")
    wq = nc.dram_tensor("wq", [E, FW], BF16, kind="ExternalInput")
    wk = nc.dram_tensor("wk", [E, FW], BF16, kind="ExternalInput")
    wv = nc.dram_tensor("wv", [E, FW], BF16, kind="ExternalInput")
    bq = nc.dram_tensor("bq", [1, FW], BF16, kind="ExternalInput")
    bk = nc.dram_tensor("bk", [1, FW], BF16, kind="ExternalInput")
    bv = nc.dram_tensor("bv", [1, FW], BF16, kind="ExternalInput")
    wo = nc.dram_tensor("wo", [FW, E], BF16, kind="ExternalInput")
    maskt = nc.dram_tensor("maskt", [S, S], BF16, kind="ExternalInput")
    out = nc.dram_tensor("out", [S, E], BF16, kind="ExternalOutput")

    with tile.TileContext(nc) as tc:
        with tc.tile_pool(name="per", bufs=1) as per, \
             tc.tile_pool(name="xp", bufs=3) as xp, \
             tc.tile_pool(name="ep", bufs=3) as ep, \
             tc.tile_pool(name="atp", bufs=64) as atp, \
             tc.tile_pool(name="mp", bufs=3) as mp, \
             tc.tile_pool(name="bcp", bufs=2) as bcp, \
             tc.tile_pool(name="outp", bufs=2) as outp:

            # ---- persistent SBUF ----
            wq_sb = per.tile([128, 8 * FW], BF16, name="wq_sb")
            wk_sb = per.tile([128, 8 * FW], BF16, name="wk_sb")
            wv_sb = per.tile([128, 8 * FW], BF16, name="wv_sb")
            wo_sb = per.tile([128, 2 * E], BF16, name="wo_sb")
            bq_sb = per.tile([1, FW], BF16, name="bq_sb")
            bk_sb = per.tile([1, FW], BF16, name="bk_sb")
            bv_sb = per.tile([1, FW], BF16, name="bv_sb")
            qht_sb = per.tile([128, 2 * S], BF16, name="qht_sb")
            kht_sb = per.tile([128, 2 * S], BF16, name="kht_sb")
            vh_sb = per.tile([128, 16 * 260], BF16, name="vh_sb")
            ctx_sb = per.tile([128, 2 * S], BF16, name="ctx_sb")
            ones_f = per.tile([1, 512], F32, name="ones_f")
            ones_b = per.tile([1, 512], BF16, name="ones_b")
            bv2_sb = per.tile([1, 512], BF16, name="bv2_sb")
            bv3_sb = per.tile([1, 512], BF16, name="bv3_sb")
            # rotating mask tiles: one [128, 16*SQW] slice per query block
            # (3 slots; block 3's tile reuses block 0's slot)
            mtiles = {}

            nc.vector.memset(ones_f[:], 1.0)
            nc.vector.tensor_copy(ones_b[:], ones_f[:])
            nc.vector.memset(vh_sb[:], 1.0)

            wdma = {"q": (wq_sb, wq, bq_sb, bq), "k": (wk_sb, wk, bk_sb, bk),
                    "v": (wv_sb, wv, bv_sb, bv)}

            def load_w(nm):
                w_sb_, w_, b_sb_, b_ = wdma[nm]
                nc.sync.dma_start(w_sb_[:].rearrange("p (c n) -> p c n", c=8),
                                  w_.ap().rearrange("(c p) n -> p c n", p=128))
                nc.sync.dma_start(b_sb_[:], b_.ap())

            def load_mask_qb(qb):
                mtiles[qb] = mp.tile([128, 16 * SQW], BF16, tag="mask",
                                     name=f"mask{qb}")
                for c in range(16):
                    nc.sync.dma_start(
                        mtiles[qb][:, c * SQW:(c + 1) * SQW],
                        maskt.ap()[c * 128:(c + 1) * 128,
                                   qb * SQW:(qb + 1) * SQW])

            # ================= phase A: projections =================
            # DMA order: q/k weights + x first (QK attention can start right
            # after the k projection), masks + v after, wo last.
            load_w("q")
            load_w("k")

            with tc.tile_pool(name="pp", bufs=1, space="PSUM") as pp:
                # q and k projections: qhT/khT [256, S] bf16 as
                # [128, fc * S] (fc = feature chunk of 128 = 2 heads)
                for nm, xdram, w_sb, b_sb, dst in (
                        ("q", xqt, wq_sb, bq_sb, qht_sb),
                        ("k", xkt, wk_sb, bk_sb, kht_sb)):
                    accs = [pp.tile([128, 512], F32, tag=f"acc{i}", name=f"acc_{nm}{i}")
                            for i in range(8)]
                    for e in range(8):
                        x_t = xp.tile([128, S], BF16, tag="x", name=f"x_{nm}{e}")
                        nc.sync.dma_start(x_t[:], xdram.ap()[e * 128:(e + 1) * 128, :])
                        for fc in range(2):
                            for sq in range(4):
                                nc.tensor.matmul(
                                    accs[fc * 4 + sq][:],
                                    w_sb[:, e * FW + fc * 128: e * FW + fc * 128 + 128],
                                    x_t[:, sq * 512:(sq + 1) * 512],
                                    start=(e == 0), stop=False)
                    for fc in range(2):
                        for sq in range(4):
                            a = accs[fc * 4 + sq]
                            nc.tensor.matmul(a[:], b_sb[0:1, fc * 128:fc * 128 + 128],
                                             ones_b[0:1, :], start=False, stop=True)
                            dstap = dst[:, fc * S + sq * 512: fc * S + sq * 512 + 512]
                            if sq % 2 == 0:
                                nc.scalar.copy(dstap, a[:])
                            else:
                                nc.vector.tensor_copy(dstap, a[:])
                # mask tiles for query block 0 only (2 MB) — attention on
                # block 0 can start as soon as these + q/k are in
                load_mask_qb(0)
                load_w("v")
                nc.vector.tensor_copy(bv2_sb[0:1, 0:128], bv_sb[0:1, 0:128])
                nc.vector.tensor_copy(bv2_sb[0:1, 128:256], bv_sb[0:1, 0:128])
                nc.vector.tensor_copy(bv2_sb[0:1, 256:384], bv_sb[0:1, 0:128])
                nc.vector.tensor_copy(bv2_sb[0:1, 384:512], bv_sb[0:1, 0:128])
                nc.vector.tensor_copy(bv3_sb[0:1, 0:128], bv_sb[0:1, 128:256])
                nc.vector.tensor_copy(bv3_sb[0:1, 128:256], bv_sb[0:1, 128:256])
                nc.vector.tensor_copy(bv3_sb[0:1, 256:384], bv_sb[0:1, 128:256])
                nc.vector.tensor_copy(bv3_sb[0:1, 384:512], bv_sb[0:1, 128:256])

            # ============ phase B + C: attention, query-block outer ============
            # Software-pipelined head of the schedule: both head-pairs' QK,
            # exp and mask-mul for query block 0 are emitted BEFORE the V
            # projection (their PV matmuls are deferred via a 64-slot at
            # pool), so the scalar engine starts the exp stream as soon as
            # the k projection lands instead of waiting for xv. The V
            # projection itself is split into two 4-bank passes (one per
            # head-pair) that live in the shared attention PSUM pool.
            # Deferred emission elsewhere: the normalize tensor-muls of
            # block i are emitted a few iterations into block i+1 (DVE never
            # stalls on the gpsimd broadcast), and phase C of query block sq
            # is emitted mid-attention of block sq+1 (its PSUM evacuations
            # never block the strict-FIFO ACT/DVE queues, and the extra
            # matmuls keep the PE warm).
            with tc.tile_pool(name="sp", bufs=2, space="PSUM") as sp, \
                 tc.tile_pool(name="cp", bufs=4, space="PSUM") as cp:
                pending_tt = []    # closures: normalize TT muls of prev block
                pending_c = []     # closures: phase C of prev query block

                def v_pass(fcx):
                    # V projection for heads 2*fcx, 2*fcx+1: 16 key chunks x
                    # 128 features in 4 one-bank accumulators.
                    accs = [cp.tile([128, 512], F32, tag="ps1", name=f"vacc{fcx}_{j}")
                            for j in range(4)]
                    bvt = bv2_sb if fcx == 0 else bv3_sb
                    for j in range(4):
                        nc.tensor.matmul(accs[j][:], ones_b[0:1, 0:128], bvt[0:1, :],
                                         start=True, stop=False, skip_group_check=True)
                    for e in range(8):
                        x_t = xp.tile([128, S], BF16, tag="x", name=f"x_v{fcx}_{e}")
                        nc.sync.dma_start(x_t[:], xvt.ap()[e * 128:(e + 1) * 128, :])
                        for j in range(4):
                            for m in range(4):
                                sk = j * 4 + m
                                nc.tensor.matmul(
                                    accs[j][:, m * 128:(m + 1) * 128],
                                    x_t[:, sk * 128:(sk + 1) * 128],
                                    wv_sb[:, e * FW + fcx * 128: e * FW + fcx * 128 + 128],
                                    start=False, stop=(e == 7), skip_group_check=True)
                    for j in range(4):
                        for m in range(4):
                            sk = j * 4 + m
                            dstap = (vh_sb[:, sk * 260 + 2 * fcx * 65:
                                           sk * 260 + 2 * fcx * 65 + 130]
                                     .rearrange("p (h z) -> p h z", h=2)[:, :, 0:D])
                            srcap = (accs[j][:, m * 128:(m + 1) * 128]
                                     .rearrange("p (h z) -> p h z", h=2))
                            if m % 2 == 0:
                                nc.scalar.copy(dstap, srcap)
                            else:
                                nc.vector.tensor_copy(dstap, srcap)

                def phase_c(sq):
                    q0 = sq * SQW
                    for qc in range(4):
                        r0 = q0 + qc * 128
                        o_t = outp.tile([128, E], BF16, tag="o", name=f"o{sq}_{qc}")
                        for i in range(2):
                            op_ps = cp.tile([128, 512], F32, tag="ps1",
                                            name=f"op{sq}_{qc}_{i}")
                            for fcc in range(2):
                                nc.tensor.matmul(
                                    op_ps[:],
                                    ctx_sb[:, fcc * S + r0: fcc * S + r0 + 128],
                                    wo_sb[:, fcc * E + i * 512: fcc * E + i * 512 + 512],
                                    start=(fcc == 0), stop=(fcc == 1))
                            dstap = o_t[:, i * 512:(i + 1) * 512]
                            if (qc + i) % 2 == 0:
                                nc.scalar.copy(dstap, op_ps[:])
                            else:
                                nc.vector.tensor_copy(dstap, op_ps[:])
                        nc.sync.dma_start(out.ap()[r0:r0 + 128, :], o_t[:])

                def qk_exp_mul(sq, fc, sk):
                    """Emit the QK pair, exp, and the two mask-muls; return
                    the two at tiles (PV may be emitted later)."""
                    q0 = sq * SQW
                    sc = sp.tile([128, 1024], F32, tag="sc", name=f"sc{sq}_{fc}_{sk}")
                    for po, half in ((0, 0), (64, 1)):
                        nc.tensor.matmul(
                            sc[:, half * 512: half * 512 + 512],
                            kht_sb[po:po + 64,
                                   fc * S + sk * 128: fc * S + sk * 128 + 128],
                            qht_sb[po:po + 64, fc * S + q0: fc * S + q0 + SQW],
                            start=True, stop=True)
                    ex = ep.tile([128, 1024], BF16, tag="ex", name=f"ex{sq}_{fc}_{sk}")
                    nc.scalar.activation(ex[:], sc[:], Exp, scale=0.125)
                    mchunk = mtiles[sq][:, sk * SQW:(sk + 1) * SQW]
                    ats = []
                    for half in (0, 1):
                        at = atp.tile([128, SQW], BF16, tag="at",
                                      name=f"at{sq}_{fc}_{sk}_{half}")
                        nc.vector.tensor_mul(
                            at[:], ex[:, half * 512: half * 512 + 512], mchunk)
                        ats.append(at)
                    return ats

                def pv(fc, sk, half, at, ctx):
                    h = fc * 2 + half
                    nc.tensor.matmul(
                        ctx[:],
                        vh_sb[:, sk * 260 + h * 65: sk * 260 + h * 65 + 65],
                        at[:],
                        start=(sk == 0), stop=(sk == 15), skip_group_check=True)

                def normalize(sq, fc, po, ctx, defer):
                    q0 = sq * SQW
                    r_f = bcp.tile([1, SQW], F32, tag="r_f", name=f"rf{sq}_{fc}_{po}")
                    nc.vector.tensor_copy(r_f[:], ctx[64:65, :])
                    r_rec = bcp.tile([1, SQW], F32, tag="r_rec",
                                     name=f"rr{sq}_{fc}_{po}")
                    nc.vector.reciprocal_approx_fast(r_rec[:], r_f[:])
                    bc_t = bcp.tile([64, SQW], F32, tag="bc", name=f"bc{sq}_{fc}_{po}")
                    nc.gpsimd.partition_broadcast(bc_t[:], r_rec[:])

                    def tt():
                        nc.vector.tensor_mul(
                            ctx_sb[po:po + 64, fc * S + q0: fc * S + q0 + SQW],
                            ctx[0:64, :], bc_t[:])
                    if defer:
                        pending_tt.append(tt)
                    else:
                        tt()

                # ---- pipelined first query block ----
                ats0 = {}
                for fc in range(2):
                    for sk in range(16):
                        ats0[(fc, sk)] = qk_exp_mul(0, fc, sk)
                v_pass(0)
                load_mask_qb(1)
                ctxA = cp.tile([65, SQW], F32, tag="ps1", name="ctxA0_0")
                ctxB = cp.tile([65, SQW], F32, tag="ps1", name="ctxB0_0")
                for sk in range(16):
                    for half, ctx in ((0, ctxA), (1, ctxB)):
                        pv(0, sk, half, ats0[(0, sk)][half], ctx)
                normalize(0, 0, 0, ctxA, defer=False)
                normalize(0, 0, 64, ctxB, defer=False)
                v_pass(1)
                nc.sync.dma_start(wo_sb[:].rearrange("p (c n) -> p c n", c=2),
                                  wo.ap().rearrange("(c p) n -> p c n", p=128))
                load_mask_qb(2)
                load_mask_qb(3)
                ctxA = cp.tile([65, SQW], F32, tag="ps1", name="ctxA0_1")
                ctxB = cp.tile([65, SQW], F32, tag="ps1", name="ctxB0_1")
                for sk in range(16):
                    for half, ctx in ((0, ctxA), (1, ctxB)):
                        pv(1, sk, half, ats0[(1, sk)][half], ctx)
                normalize(0, 1, 0, ctxA, defer=True)
                normalize(0, 1, 64, ctxB, defer=True)
                pending_c.append(lambda: phase_c(0))
                del ats0

                # ---- steady-state blocks ----
                for sq in range(1, NSQ):
                    for fc in range(2):
                        ctxA = cp.tile([65, SQW], F32, tag="ps1", name=f"ctxA{sq}_{fc}")
                        ctxB = cp.tile([65, SQW], F32, tag="ps1", name=f"ctxB{sq}_{fc}")
                        for sk in range(16):
                            atA, atB = qk_exp_mul(sq, fc, sk)
                            pv(fc, sk, 0, atA, ctxA)
                            pv(fc, sk, 1, atB, ctxB)
                            if sk == 2:
                                for f in pending_tt:
                                    f()
                                pending_tt.clear()
                            if sk == 6:
                                for f in pending_c:
                                    f()
                                pending_c.clear()
                        normalize(sq, fc, 0, ctxA, defer=True)
                        normalize(sq, fc, 64, ctxB, defer=True)
                    pending_c.append(lambda sq=sq: phase_c(sq))

                for f in pending_tt:
                    f()
                for f in pending_c:
                    f()

    nc.compile()
    return nc


_CACHE = {}


def _get_nc():
    if "nc" not in _CACHE:
        _CACHE["nc"] = build_nc()
    return _CACHE["nc"]


class _FastRunner:
    """Persistent fast-dispatch shard_map executable for the compiled Bass
    module: one AOT compile, then C++ fast-path dispatch per call (the
    effectful jit path costs ~0.7ms/call in host dispatch)."""

    def __init__(self, nc):
        import jax
        from jax.sharding import Mesh, PartitionSpec, NamedSharding
        from jax.experimental.shard_map import shard_map
        from concourse.bass2jax import (
            _bass_exec_p, partition_id_tensor, install_neuronx_cc_hook,
            fast_dispatch_compile,
        )

        install_neuronx_cc_hook()
        self.jax = jax
        pname = nc.partition_id_tensor.name if nc.partition_id_tensor else None
        in_names, out_names, out_avals, zero_shapes = [], [], [], []
        for alloc in nc.m.functions[0].allocations:
            if not isinstance(alloc, mybir.MemoryLocationSet):
                continue
            name = alloc.memorylocations[0].name
            if alloc.kind == "ExternalInput":
                if name != pname:
                    in_names.append(name)
            elif alloc.kind == "ExternalOutput":
                shape = tuple(alloc.tensor_shape)
                dtype = mybir.dt.np(alloc.dtype)
                out_names.append(name)
                out_avals.append(jax.core.ShapedArray(shape, dtype))
                zero_shapes.append((shape, dtype))
        self.in_names = in_names
        self.out_names = out_names
        self.zero_shapes = zero_shapes
        n_params = len(in_names)
        all_in = in_names + out_names + ([pname] if pname else [])
        donate = tuple(range(n_params, n_params + len(out_avals)))

        def _body(*args):
            operands = list(args)
            if pname is not None:
                operands.append(partition_id_tensor())
            return tuple(_bass_exec_p.bind(
                *operands, out_avals=tuple(out_avals),
                in_names=tuple(all_in), out_names=tuple(out_names),
                lowering_input_output_aliases=(),
                sim_require_finite=True, sim_require_nnan=True, nc=nc))

        devices = jax.devices()[:NCORES]
        mesh = Mesh(np.asarray(devices), ("core",))
        self.sharding = NamedSharding(mesh, PartitionSpec("core"))
        n_outs = len(out_avals)
        fn = shard_map(
            _body, mesh=mesh,
            in_specs=(PartitionSpec("core"),) * (n_params + n_outs),
            out_specs=(PartitionSpec("core"),) * n_outs, check_rep=False)
        self._fn = fn
        self._fast = None
        self._fast_dispatch_compile = fast_dispatch_compile
        self._donate = donate

    def _compile_fast(self, concat_in):
        jax = self.jax
        avals = [jax.ShapeDtypeStruct(a.shape, a.dtype, sharding=self.sharding)
                 for a in concat_in]
        avals += [jax.ShapeDtypeStruct((NCORES * s[0], *s[1:]), d,
                                       sharding=self.sharding)
                  for (s, d) in self.zero_shapes]
        self._fast = self._fast_dispatch_compile(
            lambda: self.jax.jit(self._fn, donate_argnums=self._donate,
                                 keep_unused=True).lower(*avals).compile())

    def zeros(self):
        return [self.jax.device_put(
            np.zeros((NCORES * s[0], *s[1:]), d), self.sharding)
            for (s, d) in self.zero_shapes]

    def run(self, in_maps):
        """Full-path execute: host arrays in, list of per-core dicts out."""
        jax = self.jax
        per_core = [[np.asarray(m[name]) for name in self.in_names]
                    for m in in_maps]
        concat_in = [jax.device_put(
            np.concatenate([per_core[c][i] for c in range(NCORES)], axis=0),
            self.sharding) for i in range(len(self.in_names))]
        if self._fast is None:
            self._compile_fast(concat_in)
        outs = self._fast(*concat_in, *self.zeros())
        res = []
        for c in range(NCORES):
            d = {}
            for i, name in enumerate(self.out_names):
                s, _ = self.zero_shapes[i]
                d[name] = np.asarray(outs[i]).reshape(NCORES, *s)[c]
            res.append(d)
        return res


def _get_runner():
    if "runner" not in _CACHE:
        _CACHE["runner"] = _FastRunner(_get_nc())
    return _CACHE["runner"]


def make_in_maps(q, k, v, mask, Wqkv, bqkv, Wout):
    bf = ml_dtypes.bfloat16
    maskt = np.ascontiguousarray(mask[0, 0].T).astype(bf)
    qT = [np.ascontiguousarray(np.asarray(q[b]).T).astype(bf) for b in range(B)]
    kT = [np.ascontiguousarray(np.asarray(k[b]).T).astype(bf) for b in range(B)]
    vT = [np.ascontiguousarray(np.asarray(v[b]).T).astype(bf) for b in range(B)]
    Wqkv = np.asarray(Wqkv)
    bqkv = np.asarray(bqkv)
    Wout = np.asarray(Wout)
    in_maps = []
    for c in range(NCORES):
        b = c // 4
        h0 = (c % 4) * HPC
        fsl = slice(h0 * D, (h0 + HPC) * D)
        in_maps.append({
            "xqt": qT[b],
            "xkt": kT[b],
            "xvt": vT[b],
            "wq": np.ascontiguousarray(Wqkv[:, 0:E][:, fsl]).astype(bf),
            "wk": np.ascontiguousarray(Wqkv[:, E:2 * E][:, fsl]).astype(bf),
            "wv": np.ascontiguousarray(Wqkv[:, 2 * E:3 * E][:, fsl]).astype(bf),
            "bq": np.ascontiguousarray(bqkv[0:E][fsl]).reshape(1, FW).astype(bf),
            "bk": np.ascontiguousarray(bqkv[E:2 * E][fsl]).reshape(1, FW).astype(bf),
            "bv": np.ascontiguousarray(bqkv[2 * E:3 * E][fsl]).reshape(1, FW).astype(bf),
            "wo": np.ascontiguousarray(Wout[fsl, :]).astype(bf),
            "maskt": maskt,
        })
    return in_maps


def gather(results, bout):
    out = np.empty((B, S, E), np.float32)
    for b in range(B):
        acc = results[4 * b]["out"].astype(np.float32)
        for c in range(4 * b + 1, 4 * b + 4):
            acc += results[c]["out"].astype(np.float32)
        out[b] = acc + np.asarray(bout)[None, :]
    return out


def kernel(q, k, v, mask, Wqkv, bqkv, Wout, bout):
    runner = _get_runner()
    in_maps = make_in_maps(q, k, v, mask, Wqkv, bqkv, Wout)
    results = runner.run(in_maps)
    return gather(results, np.asarray(bout))

